# revision 28
# baseline (speedup 1.0000x reference)
"""Trainium2 Bass kernel for nn_EncodingModule2d (vq_codebook).

Pipeline per batch item (pure data parallel, 1 item per NeuronCore, 8 cores):
  stem:   s = conv_w @ x  (1x1 conv as 256x256 matmul over 4096 positions)
          y = relu(BN2(s))                          -- BN folded into weights on host
  vq:     dist2[n,k] = |y_n|^2 - 2<y_n, c_k> + |c_k|^2
          a = softmax_k(scales_k * dist2)
          agg[k,:] = sum_n a[n,k] (y_n - c_k)
  post:   z = mean_k relu(BN1(agg))                 -- BN folded on host
          g = sigmoid(head_w @ z + head_b)
  out:    relu(x + x * g) = relu(x * (1 + g))

Schedule: fully pipelined per pair of 512-column n-slices.  For pair p the PE
stream is [stem B s0, stem A s0, stem B s1, stem A s1, logits pK, logits
transposes, agg(p-1)]; softmax for pair p runs on ACT/DVE under pair p+1's PE
work, and the aggregation matmuls for pair p are slotted into the PE stream one
pair later.  This keeps the PE continuously busy (full 2.4 GHz p-state) and
leaves only an 8-matmul aggregation + BN1/head tail after the last softmax.

The kernel computes the stem in BOTH (d,n) and (n,d) layouts (stem B matmuls +
PE transposes) because the distance matmul contracts over d while the
aggregation matmul contracts over n.

dtype strategy: float32r (1 cyc/row on the PE when the moving dim >= 256) for
all the big matmuls; fp32r matmuls need an even moving size, hence the
258-wide aggregation rhs (256 y columns + ones column + pad column).
"""

import os
import sys

for _p in ("/opt/trn_rl_repo",):
    if _p not in sys.path and os.path.isdir(_p):
        sys.path.insert(0, _p)

from contextlib import ExitStack

import numpy as np

import concourse.bass as bass
import concourse.tile as tile
from concourse import bacc, mybir
from concourse.bass_utils import run_bass_kernel_spmd
from concourse.masks import make_identity

F32 = mybir.dt.float32
F32R = mybir.dt.float32r
BF16 = mybir.dt.bfloat16
AF = mybir.ActivationFunctionType
ALU = mybir.AluOpType

B, D, H, W, K = 8, 256, 64, 64, 32
HW = H * W          # 4096 spatial positions
NB = D // 128       # 2 channel blocks of 128
NS = HW // 512      # 8 n-slices of 512
NP = NS // 2        # 4 slice pairs
NCH = HW // 128     # 32 n-chunks of 128
CW = D + 2          # y_nd chunk width: 256 y + ones + pad (fp32r needs even N)
EPS = 1e-5
N_CORES = 8


def _strided_cols(t, start, step, count, width):
    """AP over columns [start + i*step : start + i*step + width) of a 2D tile."""
    a = t[:, start : start + 1]
    return bass.AP(tensor=a.tensor, offset=a.offset, ap=[a.ap[0], [step, count], [1, width]])


def _build_program(has_bias2):
    nc = bacc.Bacc("TRN2", target_bir_lowering=False, debug=False, num_devices=N_CORES)

    x_d = nc.dram_tensor("x", [D, HW], F32R, kind="ExternalInput").ap()
    # weights split so wT can stream ahead of the x pieces on its queue
    prw_d = nc.dram_tensor("prw", [D, D], F32R, kind="ExternalInput").ap()
    prr_d = nc.dram_tensor("prr", [D, K], F32R, kind="ExternalInput").ap()       # ct2
    pf_d = nc.dram_tensor("packf", [D, D + 4], F32, kind="ExternalInput").ap()   # [hwT | chv]
    sm_d = nc.dram_tensor("small", [K, D + 2], F32, kind="ExternalInput").ap()   # [centers | scc | pad]
    ssc_d = nc.dram_tensor("ssc", [1, K], F32R, kind="ExternalInput").ap()
    out_d = nc.dram_tensor("out", [D, HW], F32, kind="ExternalOutput").ap()

    with tile.TileContext(nc) as tc, ExitStack() as ctx:
        sb = ctx.enter_context(tc.tile_pool(name="sb", bufs=1))

        x_sb = sb.tile([128, NB, HW], F32R)
        wT = sb.tile([128, NB, D], F32R)
        prr = sb.tile([128, NB, K], F32R)
        packf = sb.tile([128, NB, D + 4], F32)
        small = sb.tile([K, D + 2], F32)
        srep = sb.tile([128, K], F32R)           # scales replicated over partitions

        # ---- DMA triggers -------------------------------------------------
        # sync queue: wT first, then x c-block 0, packf last;
        # scalar queue: prr first, then x c-block 1;
        # gpsimd software DGE: the tiny tensors.
        nc.sync.dma_start(wT[:, 0, :], prw_d[0:128, :])
        nc.scalar.dma_start(wT[:, 1, :], prw_d[128:256, :])
        pieces = [(0, 256), (256, 512), (512, 1024), (1024, 2048), (2048, 3072), (3072, 4096)]
        for q, (lo, hi) in enumerate(pieces):
            cs = slice(lo, hi)
            nc.sync.dma_start(x_sb[:, 0, cs], x_d[0:128, cs])
            nc.scalar.dma_start(x_sb[:, 1, cs], x_d[128:256, cs])
            if q == 0:
                nc.sync.dma_start(prr[:], prr_d.rearrange("(c p) m -> p c m", p=128))
                nc.sync.dma_start(srep[:], ssc_d.partition_broadcast(128))
                nc.sync.dma_start(small[:], sm_d)
        nc.sync.dma_start(packf[:], pf_d.rearrange("(c p) m -> p c m", p=128))

        ct2 = prr[:, :, 0:K]                     # -2*scales[k]*centers[k,d]
        sc2col = small[0:K, D : D + 1]           # scales[k]*|c_k|^2 (bias column)
        hwT = packf[:, :, 0:D]                   # head_w.T / K
        chv = packf[:, :, D : D + 4]             # [bias2, s1, bb1, -head_b]
        ckd = small[0:K, 0:D]                    # centers (k,d)

        ident = sb.tile([32, 32], F32)
        make_identity(nc, ident[:])

        # ---- big intermediates --------------------------------------------
        y_dn = sb.tile([128, NB, HW], F32R)      # relu(W'x): d on partitions
        y_dnb = sb.tile([128, NB, HW], BF16)     # bf16 copy feeding DMA transposes
        y_ndb = sb.tile([128, NB, HW], BF16)     # (n, d) layout, c-plane major
        ysq = sb.tile([128, NB, 4, 512], F32R)   # y^2 ring (4 groups of 512)
        lkn = sb.tile([32, 2, 512], F32)         # logits (k, n) ring (2 groups)
        l_nk = sb.tile([128, 2, 4, K], F32)      # psL staged to SBUF (2 groups)
        esub = sb.tile([128, 2, 4, K], F32)      # logits - max ring (2 groups)
        e_sb = sb.tile([128, 2, 4, K], F32)      # exp ring (2 groups)
        a_sb = sb.tile([128, 4, 4, K], BF16)     # softmax weights ring (4 groups)
        maxt = sb.tile([128, NCH], F32)
        nmax = sb.tile([128, NCH], F32)
        sumt = sb.tile([128, NCH], F32)
        rcp = sb.tile([128, NCH], F32)
        out_sb = sb.tile([128, NB, HW], F32)

        onesb = sb.tile([128, 2], BF16)          # moving operand for rowsum MMs
        nc.vector.memset(onesb[:], 1.0)
        # 0/1 selector folding the 4 chunk-groups of pr2 into one k row-sum
        ident4 = sb.tile([128, K], F32)
        for q4 in range(4):
            make_identity(nc, ident4[q4 * K : (q4 + 1) * K, :])

        # warm the exp table on ACT early (hidden under the x DMA)
        warm = sb.tile([128, 1], F32)
        nc.vector.memset(warm[:], 0.0)
        nc.scalar.activation(warm[:], warm[:], AF.Exp)

        psB = ctx.enter_context(tc.tile_pool(name="psB", bufs=2, space="PSUM"))
        psA = ctx.enter_context(tc.tile_pool(name="psA", bufs=1, space="PSUM"))
        psK = ctx.enter_context(tc.tile_pool(name="psK", bufs=1, space="PSUM"))
        psL = ctx.enter_context(tc.tile_pool(name="psL", bufs=2, space="PSUM"))
        psG = ctx.enter_context(tc.tile_pool(name="psG", bufs=1, space="PSUM"))
        psR = ctx.enter_context(tc.tile_pool(name="psR", bufs=1, space="PSUM"))

        # HAM warm-up: keep the PE busy on dummy transposes of the small
        # identity while x streams in, so the clock gate is ramping when the
        # real matmuls start.
        def emit_warm(n):
            for i in range(n):
                pW = psA.tile([32, 32], F32, name="warm", tag="pA")
                nc.tensor.transpose(pW[:], ident[:], ident[:])

        emit_warm(14)

        pagg_t = psG.tile([32, D], F32, name="pagg")
        pr2_t = psR.tile([128, 2], F32, name="pr2")
        pagg = pagg_t[:]                         # sum_n a (y - c) accumulator
        pr2 = pr2_t[:]                           # per-(chunk%4, k) a row-sums

        def emit_softmax(g):
            """softmax over k for group g's 4 chunks; psL half -> a_sb ring."""
            p = g // 2
            lp = pairL[p][:, (g % 2) * 4 * K : (g % 2 + 1) * 4 * K].rearrange(
                "q (c k) -> q c k", k=K)
            gs = slice(4 * g, 4 * g + 4)
            nc.scalar.activation(l_nk[:, g % 2], lp, AF.Identity)
            nc.vector.tensor_reduce(out=maxt[:, gs], in_=l_nk[:, g % 2],
                                    axis=mybir.AxisListType.X, op=ALU.max)
            mb = maxt[:, gs].rearrange("p (c u) -> p c u", u=1).broadcast_to((128, 4, K))
            nc.vector.tensor_tensor(out=esub[:, g % 2], in0=l_nk[:, g % 2], in1=mb,
                                    op=ALU.subtract)
            nc.scalar.activation(e_sb[:, g % 2], esub[:, g % 2], AF.Exp)
            nc.vector.tensor_reduce(out=sumt[:, gs], in_=e_sb[:, g % 2],
                                    axis=mybir.AxisListType.X, op=ALU.add)
            nc.vector.reciprocal(rcp[:, gs], sumt[:, gs])
            rb = rcp[:, gs].rearrange("p (c u) -> p c u", u=1).broadcast_to((128, 4, K))
            nc.vector.tensor_tensor(out=a_sb[:, g % 4], in0=e_sb[:, g % 2],
                                    in1=rb, op=ALU.mult)

        def emit_agg(g):
            """aggregation matmuls for group g's 4 chunks into the accumulator,
            plus a row-sum matmul (the 4 a-chunks side-by-side vs ones)."""
            for ci in range(4):
                j = 4 * g + ci
                nc.tensor.matmul(
                    pagg,
                    a_sb[:, g % 4, ci, :],
                    y_ndb[:, :, j * 128 : (j + 1) * 128],
                    start=(j == 0), stop=(j == NCH - 1))
            nc.tensor.matmul(
                pr2, a_sb[:, g % 4], onesb[:],
                start=(g == 0), stop=(g == 7))

        # main loop: 16 micro-slices of 256 columns; logits per group of 2
        # micro-slices (512 cols); softmax+agg per pair of groups (1024 cols)
        pairL = []
        for g in range(8):
            p = g // 2
            if g % 2 == 0:
                pL = psL.tile([128, 8 * K], F32, name=f"pL{p % 2}", tag="pL")
                pairL.append(pL)
            for ui in range(2):
                u = 2 * g + ui
                us = slice(u * 256, (u + 1) * 256)
                # --- stem B: y_dn[:, :, us] = relu(wT' x)  (one psum bank
                #     holds both o-blocks; single copy per micro-slice)
                pB = psB.tile([128, NB, 256], F32, name="pB", tag="pB")
                for o in range(NB):
                    for c in range(NB):
                        nc.tensor.matmul(
                            pB[:, o, :],
                            wT[:, c, o * 128 : (o + 1) * 128],
                            x_sb[:, c, us],
                            start=(c == 0),
                            stop=(c == NB - 1),
                        )
                dst = y_dn[:, :, us]
                if u % 2 == 0:
                    if has_bias2:
                        nc.scalar.activation(dst, pB[:], AF.Relu, bias=chv[:, :, 0:1])
                    else:
                        nc.scalar.activation(dst, pB[:], AF.Relu)
                else:
                    if has_bias2:
                        nc.vector.tensor_scalar(
                            out=dst, in0=pB[:], scalar1=chv[:, :, 0:1],
                            scalar2=0.0, op0=ALU.add, op1=ALU.max)
                    else:
                        nc.vector.tensor_scalar_max(out=dst, in0=pB[:], scalar1=0.0)

                # --- squares for the distance matmuls (spread 2:1:1 over
                #     GpSimd / ACT / DVE per pair)
                sq_dst = ysq[:, :, g % 4, ui * 256 : (ui + 1) * 256]
                if u % 4 in (0, 2):
                    nc.gpsimd.tensor_mul(sq_dst, y_dn[:, :, us], y_dn[:, :, us])
                elif u % 4 == 1:
                    nc.scalar.activation(sq_dst, y_dn[:, :, us], AF.Square)
                else:
                    for c in range(NB):
                        nc.vector.tensor_tensor(
                            out=ysq[:, c, g % 4, ui * 256 : (ui + 1) * 256],
                            in0=y_dn[:, c, us], in1=y_dn[:, c, us], op=ALU.mult)

                # --- bf16 copy of y for the DMA-engine transposes ----------
                if u % 2 == 0:
                    nc.vector.tensor_copy(y_dnb[:, :, us], y_dn[:, :, us])
                else:
                    nc.scalar.activation(y_dnb[:, :, us], y_dn[:, :, us],
                                         AF.Identity)

            # --- logits for the group, (k, n) orientation -----------------
            #   lkn[k, n] = sum_d ct2[d,k] y[d,n] + sum_d srep[d,k] ysq[d,n]
            #             = -2 s_k <y_n, c_k> + s_k |y_n|^2
            # + scc[k] added as a per-partition bias in the PSUM->SBUF copy.
            gsl = slice(g * 512, (g + 1) * 512)
            pKt = psK.tile([32, 512], F32, name="pK", tag="pK")
            nc.tensor.matmul(pKt[:], ct2[:, 0, :], y_dn[:, 0, gsl],
                             start=True, stop=False)
            nc.tensor.matmul(pKt[:], ct2[:, 1, :], y_dn[:, 1, gsl],
                             start=False, stop=False)
            nc.tensor.matmul(pKt[:], srep[:], ysq[:, 0, g % 4, :],
                             start=False, stop=False)
            nc.tensor.matmul(pKt[:], srep[:], ysq[:, 1, g % 4, :],
                             start=False, stop=True)
            nc.scalar.activation(lkn[:, g % 2, :], pKt[:], AF.Identity,
                                 bias=sc2col[:])
            # transpose 32x128 blocks into the pair's (n, k) psum tile
            for jj in range(4):
                nc.tensor.transpose(
                    pairL[p][:, (4 * (g % 2) + jj) * K : (4 * (g % 2) + jj + 1) * K],
                    lkn[:, g % 2, jj * 128 : (jj + 1) * 128], ident[:])

            # --- stem A via the DMA crossbar
            #     (the XBAR writes chunk-major: y_ndb[p, c, 128j + m] =
            #      y[n = 128j + p, d = 128c + m])
            gsl0 = slice(g * 512, (g + 1) * 512)
            for c in range(NB):
                nc.sync.dma_start(
                    y_ndb[:, c, gsl0].rearrange("p (j m) -> p j m", j=4),
                    y_dnb[:, c, gsl0], transpose=True)
            # two-group-lagged aggregation: its softmax ran under the last
            # two groups' PE work, so the PE never waits on it
            if g >= 2:
                emit_agg(g - 2)
            emit_softmax(g)
        emit_agg(6)
        emit_warm(16)
        emit_agg(7)
        emit_warm(3)

        # ---- tail: BN1 + mean + head + gate -------------------------------
        # rowsum_a[k] = sum over the 4 chunk-groups of pr2 (one matmul against
        # the stacked-identity selector)
        pr2s = sb.tile([128, 2], F32)
        nc.scalar.activation(pr2s[:], pr2, AF.Identity)
        prs = psA.tile([32, 2], F32, name="prs", tag="pA")
        nc.tensor.matmul(prs[:], ident4[:], pr2s[:], start=True, stop=True)
        # agg[k,d] = pagg[k,d] - rowsum_a[k] * centers[k,d]
        rsc = sb.tile([32, D], F32)
        nc.vector.tensor_scalar_mul(out=rsc[:], in0=ckd[:], scalar1=prs[:, 0:1])
        agg_sb = sb.tile([32, D], F32)
        nc.vector.tensor_tensor(out=agg_sb[:], in0=pagg, in1=rsc[:], op=ALU.subtract)

        # BN1 + relu + mean over k  ->  z per d-block
        z_t = sb.tile([128, NB], F32)
        t_sb = sb.tile([128, NB, K], F32)
        for b in range(NB):
            pT = psA.tile([128, 32], F32, name="pT", tag="pA")
            nc.tensor.transpose(pT[:], agg_sb[:, b * 128 : (b + 1) * 128], ident[:])
            nc.scalar.activation(t_sb[:, b, :], pT[:], AF.Relu,
                                 bias=chv[:, b, 2:3], scale=chv[:, b, 1:2])
            nc.vector.tensor_reduce(out=z_t[:, b : b + 1],
                                    in_=t_sb[:, b, :],
                                    axis=mybir.AxisListType.X, op=ALU.add)

        emit_warm(4)
        # head: gate = 1 + sigmoid(head_w @ z + head_b)
        gate = sb.tile([128, NB], F32)
        eg = sb.tile([128, NB], F32)
        for o in range(NB):
            pH = psA.tile([128, 1], F32, name="pH", tag="pA")
            for c in range(NB):
                nc.tensor.matmul(pH[:], hwT[:, c, o * 128 : (o + 1) * 128],
                                 z_t[:, c : c + 1],
                                 start=(c == 0), stop=(c == NB - 1))
            # exp(-(v + head_b)) ; gate = 1 + 1/(1+e)
            nc.scalar.activation(eg[:, o : o + 1], pH[:], AF.Exp,
                                 bias=chv[:, o, 3:4], scale=-1.0)
        nc.vector.tensor_scalar_add(out=eg[:], in0=eg[:], scalar1=1.0)
        nc.vector.reciprocal(gate[:], eg[:])
        nc.vector.tensor_scalar_add(out=gate[:], in0=gate[:], scalar1=1.0)

        # gating: out = relu(x * gate[d]) ; stream out per 1024-col piece,
        # computed on ACT/DVE, written via both hw DMA queues
        qeng = {0: nc.sync, 1: nc.scalar}
        ceng = [nc.scalar, nc.vector, nc.vector, nc.vector,
                nc.vector, nc.vector, nc.vector, nc.scalar]
        idx = 0
        for q in range(4):
            for o in range(NB):
                cs = slice(q * 1024, (q + 1) * 1024)
                eng = ceng[idx]
                idx += 1
                if eng is nc.scalar:
                    nc.scalar.activation(out_sb[:, o, cs], x_sb[:, o, cs],
                                         AF.Relu, scale=gate[:, o : o + 1])
                else:
                    eng.tensor_scalar(out=out_sb[:, o, cs], in0=x_sb[:, o, cs],
                                      scalar1=gate[:, o : o + 1], scalar2=0.0,
                                      op0=ALU.mult, op1=ALU.max)
                qeng[o].dma_start(out_d[o * 128 : (o + 1) * 128, cs], out_sb[:, o, cs])

    nc.compile()
    return nc


_PROGRAM_CACHE = {}


def _get_program(has_bias2):
    key = bool(has_bias2)
    if key not in _PROGRAM_CACHE:
        _PROGRAM_CACHE[key] = _build_program(key)
    return _PROGRAM_CACHE[key]


def _host_params(conv_w, bn2_g, bn2_b, bn2_m, bn2_v, centers, scales,
                 bn1_g, bn1_b, bn1_m, bn1_v, head_w, head_b):
    scale2 = bn2_g / np.sqrt(bn2_v + EPS)
    wT = (conv_w * scale2[:, None]).T.astype(np.float32).copy()      # (c, o)
    bias2 = (bn2_b - bn2_m * scale2).astype(np.float32)
    ct2 = (-2.0 * scales[None, :] * centers.T).astype(np.float32)    # (d, k)
    c2 = (centers * centers).sum(axis=1)
    ssc = scales.reshape(1, K).astype(np.float32)                    # (1, k)
    scc = (scales * c2).astype(np.float32)                           # (k,)
    s1 = bn1_g / np.sqrt(bn1_v + EPS)
    bb1 = bn1_b - bn1_m * s1
    chv = np.stack([bias2, s1.astype(np.float32), bb1.astype(np.float32),
                    (-head_b).astype(np.float32)], axis=1).astype(np.float32)  # (d, 4)
    hwT = (head_w.T / np.float32(K)).astype(np.float32)              # (d, o)
    prr = np.ascontiguousarray(ct2)
    packf = np.ascontiguousarray(np.concatenate([hwT, chv], axis=1))  # (d, 260)
    small = np.zeros((K, D + 2), np.float32)
    small[:, 0:D] = centers
    small[:, D] = scc
    return wT, prr, packf, small, ssc, bias2


def _ensure_profile_hook():
    """Register the axon NTFF profile hook if the image lacks antenv.axon_hooks."""
    import types

    if "antenv.axon_hooks" in sys.modules:
        return
    try:
        import antenv

        mod = types.ModuleType("antenv.axon_hooks")
        _hook = [None]
        mod.set_axon_ntff_profile_hook = lambda h: _hook.__setitem__(0, h)
        mod.get_axon_ntff_profile_hook = lambda: _hook[0]
        sys.modules["antenv.axon_hooks"] = mod
        antenv.axon_hooks = mod
        from trn_agent_boot.trn_boot import _ntff_profile_via_ctypes

        mod.set_axon_ntff_profile_hook(
            _ntff_profile_via_ctypes("/opt/axon/libaxon_pjrt.so"))
        import concourse.bass_utils as _bu

        _bu.upload_artifacts = lambda d: d  # no artifact store in this container
    except Exception as e:  # profiling is best-effort
        print(f"profile hook setup failed: {e}", file=sys.stderr)


def kernel(x, conv_w, bn2_g, bn2_b, bn2_m, bn2_v, centers, scales,
           bn1_g, bn1_b, bn1_m, bn1_v, head_w, head_b):
    x = np.ascontiguousarray(np.asarray(x, dtype=np.float32))
    wT, prr, packf, small, ssc, bias2 = _host_params(
        np.asarray(conv_w, np.float32), np.asarray(bn2_g, np.float32),
        np.asarray(bn2_b, np.float32), np.asarray(bn2_m, np.float32),
        np.asarray(bn2_v, np.float32), np.asarray(centers, np.float32),
        np.asarray(scales, np.float32), np.asarray(bn1_g, np.float32),
        np.asarray(bn1_b, np.float32), np.asarray(bn1_m, np.float32),
        np.asarray(bn1_v, np.float32), np.asarray(head_w, np.float32),
        np.asarray(head_b, np.float32))
    has_bias2 = bool(np.abs(bias2).max() > 0)
    nc = _get_program(has_bias2)

    shared = {
        "prw": wT, "prr": prr, "packf": packf, "small": small, "ssc": ssc,
    }
    in_maps = [dict(shared, x=x[b].reshape(D, HW)) for b in range(N_CORES)]

    trace = bool(int(os.environ.get("KERNEL_TRACE", "0")))
    kwargs = {}
    if trace:
        _ensure_profile_hook()
        tdir = os.environ.get("KERNEL_TRACE_DIR")
        if tdir:
            os.makedirs(tdir, exist_ok=True)
            kwargs["tmpdir"] = tdir
    res = run_bass_kernel_spmd(nc, in_maps, list(range(N_CORES)), trace=trace, **kwargs)
    if trace:
        kernel.last_exec_time_ns = res.exec_time_ns
        kernel.last_results = res
    out = np.stack([res.results[b]["out"].reshape(D, H, W) for b in range(N_CORES)])
    return out.astype(np.float32)


# revision 30
# speedup vs baseline: 1.1582x; 1.1582x over previous
"""Trainium2 Bass kernel for nn_EncodingModule2d (vq_codebook).

Pipeline per batch item (pure data parallel, 1 item per NeuronCore, 8 cores):
  stem:   s = conv_w @ x  (1x1 conv as 256x256 matmul over 4096 positions)
          y = relu(BN2(s))                          -- BN folded into weights on host
  vq:     dist2[n,k] = |y_n|^2 - 2<y_n, c_k> + |c_k|^2
          a = softmax_k(scales_k * dist2)
          agg[k,:] = sum_n a[n,k] (y_n - c_k)
  post:   z = mean_k relu(BN1(agg))                 -- BN folded on host
          g = sigmoid(head_w @ z + head_b)
  out:    relu(x + x * g) = relu(x * (1 + g))

Schedule: fully pipelined per pair of 512-column n-slices.  For pair p the PE
stream is [stem B s0, stem A s0, stem B s1, stem A s1, logits pK, logits
transposes, agg(p-1)]; softmax for pair p runs on ACT/DVE under pair p+1's PE
work, and the aggregation matmuls for pair p are slotted into the PE stream one
pair later.  This keeps the PE continuously busy (full 2.4 GHz p-state) and
leaves only an 8-matmul aggregation + BN1/head tail after the last softmax.

The kernel computes the stem in BOTH (d,n) and (n,d) layouts (stem B matmuls +
PE transposes) because the distance matmul contracts over d while the
aggregation matmul contracts over n.

dtype strategy: float32r (1 cyc/row on the PE when the moving dim >= 256) for
all the big matmuls; fp32r matmuls need an even moving size, hence the
258-wide aggregation rhs (256 y columns + ones column + pad column).
"""

import os
import sys

for _p in ("/opt/trn_rl_repo",):
    if _p not in sys.path and os.path.isdir(_p):
        sys.path.insert(0, _p)

from contextlib import ExitStack

import numpy as np

import concourse.bass as bass
import concourse.tile as tile
from concourse import bacc, mybir
from concourse.bass_utils import run_bass_kernel_spmd
from concourse.masks import make_identity

F32 = mybir.dt.float32
F32R = mybir.dt.float32r
BF16 = mybir.dt.bfloat16
AF = mybir.ActivationFunctionType
ALU = mybir.AluOpType

B, D, H, W, K = 8, 256, 64, 64, 32
HW = H * W          # 4096 spatial positions
NB = D // 128       # 2 channel blocks of 128
NS = HW // 512      # 8 n-slices of 512
NP = NS // 2        # 4 slice pairs
NCH = HW // 128     # 32 n-chunks of 128
CW = D + 2          # y_nd chunk width: 256 y + ones + pad (fp32r needs even N)
EPS = 1e-5
N_CORES = 8


def _strided_cols(t, start, step, count, width):
    """AP over columns [start + i*step : start + i*step + width) of a 2D tile."""
    a = t[:, start : start + 1]
    return bass.AP(tensor=a.tensor, offset=a.offset, ap=[a.ap[0], [step, count], [1, width]])


def _build_program(has_bias2):
    nc = bacc.Bacc("TRN2", target_bir_lowering=False, debug=False, num_devices=N_CORES)

    x_d = nc.dram_tensor("x", [D, HW], F32R, kind="ExternalInput").ap()
    # weights split so wT can stream ahead of the x pieces on its queue
    prw_d = nc.dram_tensor("prw", [D, D], F32R, kind="ExternalInput").ap()
    prr_d = nc.dram_tensor("prr", [D, K], F32R, kind="ExternalInput").ap()       # ct2
    pf_d = nc.dram_tensor("packf", [D, D + 4], F32, kind="ExternalInput").ap()   # [hwT | chv]
    sm_d = nc.dram_tensor("small", [K, D + 2], F32, kind="ExternalInput").ap()   # [centers | scc | pad]
    ssc_d = nc.dram_tensor("ssc", [1, K], F32R, kind="ExternalInput").ap()
    out_d = nc.dram_tensor("out", [D, HW], F32, kind="ExternalOutput").ap()

    with tile.TileContext(nc) as tc, ExitStack() as ctx:
        sb = ctx.enter_context(tc.tile_pool(name="sb", bufs=1))

        x_sb = sb.tile([128, NB, HW], F32R)
        wT = sb.tile([128, NB, D], F32R)
        prr = sb.tile([128, NB, K], F32R)
        packf = sb.tile([128, NB, D + 4], F32)
        small = sb.tile([K, D + 2], F32)
        srep = sb.tile([128, K], F32R)           # scales replicated over partitions

        # ---- DMA triggers -------------------------------------------------
        # sync queue: wT first, then x c-block 0, packf last;
        # scalar queue: prr first, then x c-block 1;
        # gpsimd software DGE: the tiny tensors.
        nc.sync.dma_start(wT[:, 0, :], prw_d[0:128, :])
        nc.scalar.dma_start(wT[:, 1, :], prw_d[128:256, :])
        pieces = [(0, 256), (256, 512), (512, 1024), (1024, 2048), (2048, 3072), (3072, 4096)]
        for q, (lo, hi) in enumerate(pieces):
            cs = slice(lo, hi)
            nc.sync.dma_start(x_sb[:, 0, cs], x_d[0:128, cs])
            nc.scalar.dma_start(x_sb[:, 1, cs], x_d[128:256, cs])
            if q == 0:
                nc.sync.dma_start(prr[:], prr_d.rearrange("(c p) m -> p c m", p=128))
                nc.sync.dma_start(srep[:], ssc_d.partition_broadcast(128))
                nc.sync.dma_start(small[:], sm_d)
        nc.sync.dma_start(packf[:], pf_d.rearrange("(c p) m -> p c m", p=128))

        ct2 = prr[:, :, 0:K]                     # -2*scales[k]*centers[k,d]
        sc2col = small[0:K, D : D + 1]           # scales[k]*|c_k|^2 (bias column)
        hwT = packf[:, :, 0:D]                   # head_w.T / K
        chv = packf[:, :, D : D + 4]             # [bias2, s1, bb1, -head_b]
        ckd = small[0:K, 0:D]                    # centers (k,d)

        ident = sb.tile([32, 32], F32)
        make_identity(nc, ident[:])

        # ---- big intermediates --------------------------------------------
        y_dn = sb.tile([128, NB, HW], F32R)      # relu(W'x): d on partitions
        y_dnb = sb.tile([128, NB, HW], BF16)     # bf16 copy feeding DMA transposes
        y_ndb = sb.tile([128, NB, HW], BF16)     # (n, d) layout, c-plane major
        ysq = sb.tile([128, NB, 4, 512], F32R)   # y^2 ring (4 groups of 512)
        lkn = sb.tile([32, 2, 512], F32)         # logits (k, n) ring (2 groups)
        l_nk = sb.tile([128, 2, 4, K], F32)      # psL staged to SBUF (2 groups)
        esub = sb.tile([128, 2, 4, K], F32)      # logits - max ring (2 groups)
        e_sb = sb.tile([128, 2, 4, K], F32)      # exp ring (2 groups)
        a_sb = sb.tile([128, 4, 4, K], BF16)     # softmax weights ring (4 groups)
        maxt = sb.tile([128, NCH], F32)
        nmax = sb.tile([128, NCH], F32)
        sumt = sb.tile([128, NCH], F32)
        rcp = sb.tile([128, NCH], F32)
        out_sb = sb.tile([128, NB, HW], F32)

        onesb = sb.tile([128, 2], BF16)          # moving operand for rowsum MMs
        nc.vector.memset(onesb[:], 1.0)
        # 0/1 selector folding the 4 chunk-groups of pr2 into one k row-sum
        ident4 = sb.tile([128, K], F32)
        for q4 in range(4):
            make_identity(nc, ident4[q4 * K : (q4 + 1) * K, :])

        # warm the exp table on ACT early (hidden under the x DMA)
        warm = sb.tile([128, 1], F32)
        nc.vector.memset(warm[:], 0.0)
        nc.scalar.activation(warm[:], warm[:], AF.Exp)

        psB = ctx.enter_context(tc.tile_pool(name="psB", bufs=2, space="PSUM"))
        psA = ctx.enter_context(tc.tile_pool(name="psA", bufs=1, space="PSUM"))
        psK = ctx.enter_context(tc.tile_pool(name="psK", bufs=1, space="PSUM"))
        psL = ctx.enter_context(tc.tile_pool(name="psL", bufs=2, space="PSUM"))
        psG = ctx.enter_context(tc.tile_pool(name="psG", bufs=1, space="PSUM"))
        psR = ctx.enter_context(tc.tile_pool(name="psR", bufs=1, space="PSUM"))

        # HAM warm-up: keep the PE busy on dummy transposes of the small
        # identity while x streams in, so the clock gate is ramping when the
        # real matmuls start.
        def emit_warm(n):
            for i in range(n):
                pW = psA.tile([32, 32], F32, name="warm", tag="pA")
                nc.tensor.transpose(pW[:], ident[:], ident[:])

        emit_warm(14)

        pagg_t = psG.tile([32, D], F32, name="pagg")
        pr2_t = psR.tile([128, 2], F32, name="pr2")
        pagg = pagg_t[:]                         # sum_n a (y - c) accumulator
        pr2 = pr2_t[:]                           # per-(chunk%4, k) a row-sums

        def emit_softmax(g):
            """softmax over k for group g's 4 chunks; psL half -> a_sb ring."""
            p = g // 2
            lp = pairL[p][:, (g % 2) * 4 * K : (g % 2 + 1) * 4 * K].rearrange(
                "q (c k) -> q c k", k=K)
            gs = slice(4 * g, 4 * g + 4)
            nc.scalar.activation(l_nk[:, g % 2], lp, AF.Identity)
            nc.vector.tensor_reduce(out=maxt[:, gs], in_=l_nk[:, g % 2],
                                    axis=mybir.AxisListType.X, op=ALU.max)
            mb = maxt[:, gs].rearrange("p (c u) -> p c u", u=1).broadcast_to((128, 4, K))
            nc.vector.tensor_tensor(out=esub[:, g % 2], in0=l_nk[:, g % 2], in1=mb,
                                    op=ALU.subtract)
            nc.scalar.activation(e_sb[:, g % 2], esub[:, g % 2], AF.Exp)
            nc.vector.tensor_reduce(out=sumt[:, gs], in_=e_sb[:, g % 2],
                                    axis=mybir.AxisListType.X, op=ALU.add)
            nc.vector.reciprocal(rcp[:, gs], sumt[:, gs])
            rb = rcp[:, gs].rearrange("p (c u) -> p c u", u=1).broadcast_to((128, 4, K))
            nc.vector.tensor_tensor(out=a_sb[:, g % 4], in0=e_sb[:, g % 2],
                                    in1=rb, op=ALU.mult)

        def emit_agg(g):
            """aggregation matmuls for group g's 4 chunks into the accumulator,
            plus a row-sum matmul (the 4 a-chunks side-by-side vs ones)."""
            for ci in range(4):
                j = 4 * g + ci
                nc.tensor.matmul(
                    pagg,
                    a_sb[:, g % 4, ci, :],
                    y_ndb[:, :, j * 128 : (j + 1) * 128],
                    start=(j == 0), stop=(j == NCH - 1))
            nc.tensor.matmul(
                pr2, a_sb[:, g % 4], onesb[:],
                start=(g == 0), stop=(g == 7))

        # main loop: 16 micro-slices of 256 columns; logits per group of 2
        # micro-slices (512 cols); softmax+agg per pair of groups (1024 cols)
        pairL = []
        for g in range(8):
            p = g // 2
            if g % 2 == 0:
                pL = psL.tile([128, 8 * K], F32, name=f"pL{p % 2}", tag="pL")
                pairL.append(pL)
            for ui in range(2):
                u = 2 * g + ui
                us = slice(u * 256, (u + 1) * 256)
                # --- stem B: y_dn[:, :, us] = relu(wT' x)  (one psum bank
                #     holds both o-blocks; single copy per micro-slice)
                pB = psB.tile([128, NB, 256], F32, name="pB", tag="pB")
                for o in range(NB):
                    for c in range(NB):
                        nc.tensor.matmul(
                            pB[:, o, :],
                            wT[:, c, o * 128 : (o + 1) * 128],
                            x_sb[:, c, us],
                            start=(c == 0),
                            stop=(c == NB - 1),
                        )
                dst = y_dn[:, :, us]
                if u % 2 == 0:
                    if has_bias2:
                        nc.scalar.activation(dst, pB[:], AF.Relu, bias=chv[:, :, 0:1])
                    else:
                        nc.scalar.activation(dst, pB[:], AF.Relu)
                else:
                    if has_bias2:
                        nc.vector.tensor_scalar(
                            out=dst, in0=pB[:], scalar1=chv[:, :, 0:1],
                            scalar2=0.0, op0=ALU.add, op1=ALU.max)
                    else:
                        nc.vector.tensor_scalar_max(out=dst, in0=pB[:], scalar1=0.0)

                # --- squares for the distance matmuls (spread 2:1:1 over
                #     GpSimd / ACT / DVE per pair)
                sq_dst = ysq[:, :, g % 4, ui * 256 : (ui + 1) * 256]
                if u % 4 in (0, 2):
                    nc.gpsimd.tensor_mul(sq_dst, y_dn[:, :, us], y_dn[:, :, us])
                elif u % 4 == 1:
                    nc.scalar.activation(sq_dst, y_dn[:, :, us], AF.Square)
                else:
                    for c in range(NB):
                        nc.vector.tensor_tensor(
                            out=ysq[:, c, g % 4, ui * 256 : (ui + 1) * 256],
                            in0=y_dn[:, c, us], in1=y_dn[:, c, us], op=ALU.mult)

                # --- bf16 copy of y for the DMA-engine transposes ----------
                if u % 2 == 0:
                    nc.vector.tensor_copy(y_dnb[:, :, us], y_dn[:, :, us])
                else:
                    nc.scalar.activation(y_dnb[:, :, us], y_dn[:, :, us],
                                         AF.Identity)

            # --- logits for the group, (k, n) orientation -----------------
            #   lkn[k, n] = sum_d ct2[d,k] y[d,n] + sum_d srep[d,k] ysq[d,n]
            #             = -2 s_k <y_n, c_k> + s_k |y_n|^2
            # + scc[k] added as a per-partition bias in the PSUM->SBUF copy.
            gsl = slice(g * 512, (g + 1) * 512)
            pKt = psK.tile([32, 512], F32, name="pK", tag="pK")
            nc.tensor.matmul(pKt[:], ct2[:, 0, :], y_dn[:, 0, gsl],
                             start=True, stop=False)
            nc.tensor.matmul(pKt[:], ct2[:, 1, :], y_dn[:, 1, gsl],
                             start=False, stop=False)
            nc.tensor.matmul(pKt[:], srep[:], ysq[:, 0, g % 4, :],
                             start=False, stop=False)
            nc.tensor.matmul(pKt[:], srep[:], ysq[:, 1, g % 4, :],
                             start=False, stop=True)
            nc.scalar.activation(lkn[:, g % 2, :], pKt[:], AF.Identity,
                                 bias=sc2col[:])
            # transpose 32x128 blocks into the pair's (n, k) psum tile
            for jj in range(4):
                nc.tensor.transpose(
                    pairL[p][:, (4 * (g % 2) + jj) * K : (4 * (g % 2) + jj + 1) * K],
                    lkn[:, g % 2, jj * 128 : (jj + 1) * 128], ident[:])

            # --- stem A via the DMA crossbar
            #     (the XBAR writes chunk-major: y_ndb[p, c, 128j + m] =
            #      y[n = 128j + p, d = 128c + m])
            gsl0 = slice(g * 512, (g + 1) * 512)
            for c in range(NB):
                nc.sync.dma_start(
                    y_ndb[:, c, gsl0].rearrange("p (j m) -> p j m", j=4),
                    y_dnb[:, c, gsl0], transpose=True)
            # two-group-lagged aggregation: its softmax ran under the last
            # two groups' PE work, so the PE never waits on it
            if g >= 2:
                emit_agg(g - 2)
            emit_softmax(g)
        emit_agg(6)
        emit_warm(14)
        emit_agg(7)
        emit_warm(3)

        # ---- tail: BN1 + mean + head + gate -------------------------------
        # rowsum_a[k] = sum over the 4 chunk-groups of pr2 (one matmul against
        # the stacked-identity selector)
        pr2s = sb.tile([128, 2], F32)
        nc.scalar.activation(pr2s[:], pr2, AF.Identity)
        emit_warm(2)
        prs = psA.tile([32, 2], F32, name="prs", tag="pA")
        nc.tensor.matmul(prs[:], ident4[:], pr2s[:], start=True, stop=True)
        emit_warm(2)
        # agg[k,d] = pagg[k,d] - rowsum_a[k] * centers[k,d]
        rsc = sb.tile([32, D], F32)
        nc.vector.tensor_scalar_mul(out=rsc[:], in0=ckd[:], scalar1=prs[:, 0:1])
        agg_sb = sb.tile([32, D], F32)
        nc.vector.tensor_tensor(out=agg_sb[:], in0=pagg, in1=rsc[:], op=ALU.subtract)

        # BN1 + relu + mean over k  ->  z per d-block
        z_t = sb.tile([128, NB], F32)
        t_sb = sb.tile([128, NB, K], F32)
        for b in range(NB):
            pT = psA.tile([128, 32], F32, name="pT", tag="pA")
            nc.tensor.transpose(pT[:], agg_sb[:, b * 128 : (b + 1) * 128], ident[:])
            nc.scalar.activation(t_sb[:, b, :], pT[:], AF.Relu,
                                 bias=chv[:, b, 2:3], scale=chv[:, b, 1:2])
            nc.vector.tensor_reduce(out=z_t[:, b : b + 1],
                                    in_=t_sb[:, b, :],
                                    axis=mybir.AxisListType.X, op=ALU.add)

        emit_warm(3)
        # head: gate = 1 + sigmoid(head_w @ z + head_b)
        gate = sb.tile([128, NB], F32)
        eg = sb.tile([128, NB], F32)
        for o in range(NB):
            pH = psA.tile([128, 1], F32, name="pH", tag="pA")
            for c in range(NB):
                nc.tensor.matmul(pH[:], hwT[:, c, o * 128 : (o + 1) * 128],
                                 z_t[:, c : c + 1],
                                 start=(c == 0), stop=(c == NB - 1))
            # exp(-(v + head_b)) ; gate = 1 + 1/(1+e)
            nc.scalar.activation(eg[:, o : o + 1], pH[:], AF.Exp,
                                 bias=chv[:, o, 3:4], scale=-1.0)
        nc.vector.tensor_scalar_add(out=eg[:], in0=eg[:], scalar1=1.0)
        nc.vector.reciprocal(gate[:], eg[:])
        nc.vector.tensor_scalar_add(out=gate[:], in0=gate[:], scalar1=1.0)

        # gating: out = relu(x * gate[d]) ; stream out per 1024-col piece,
        # computed on ACT/DVE, written via both hw DMA queues
        qeng = {0: nc.sync, 1: nc.scalar}
        ceng = [nc.scalar, nc.vector, nc.vector, nc.vector,
                nc.vector, nc.vector, nc.vector, nc.scalar]
        idx = 0
        for q in range(4):
            for o in range(NB):
                cs = slice(q * 1024, (q + 1) * 1024)
                eng = ceng[idx]
                idx += 1
                if eng is nc.scalar:
                    nc.scalar.activation(out_sb[:, o, cs], x_sb[:, o, cs],
                                         AF.Relu, scale=gate[:, o : o + 1])
                else:
                    eng.tensor_scalar(out=out_sb[:, o, cs], in0=x_sb[:, o, cs],
                                      scalar1=gate[:, o : o + 1], scalar2=0.0,
                                      op0=ALU.mult, op1=ALU.max)
                qeng[o].dma_start(out_d[o * 128 : (o + 1) * 128, cs], out_sb[:, o, cs])

    nc.compile()
    return nc


_PROGRAM_CACHE = {}


def _get_program(has_bias2):
    key = bool(has_bias2)
    if key not in _PROGRAM_CACHE:
        _PROGRAM_CACHE[key] = _build_program(key)
    return _PROGRAM_CACHE[key]


def _host_params(conv_w, bn2_g, bn2_b, bn2_m, bn2_v, centers, scales,
                 bn1_g, bn1_b, bn1_m, bn1_v, head_w, head_b):
    scale2 = bn2_g / np.sqrt(bn2_v + EPS)
    wT = (conv_w * scale2[:, None]).T.astype(np.float32).copy()      # (c, o)
    bias2 = (bn2_b - bn2_m * scale2).astype(np.float32)
    ct2 = (-2.0 * scales[None, :] * centers.T).astype(np.float32)    # (d, k)
    c2 = (centers * centers).sum(axis=1)
    ssc = scales.reshape(1, K).astype(np.float32)                    # (1, k)
    scc = (scales * c2).astype(np.float32)                           # (k,)
    s1 = bn1_g / np.sqrt(bn1_v + EPS)
    bb1 = bn1_b - bn1_m * s1
    chv = np.stack([bias2, s1.astype(np.float32), bb1.astype(np.float32),
                    (-head_b).astype(np.float32)], axis=1).astype(np.float32)  # (d, 4)
    hwT = (head_w.T / np.float32(K)).astype(np.float32)              # (d, o)
    prr = np.ascontiguousarray(ct2)
    packf = np.ascontiguousarray(np.concatenate([hwT, chv], axis=1))  # (d, 260)
    small = np.zeros((K, D + 2), np.float32)
    small[:, 0:D] = centers
    small[:, D] = scc
    return wT, prr, packf, small, ssc, bias2


def _ensure_profile_hook():
    """Register the axon NTFF profile hook if the image lacks antenv.axon_hooks."""
    import types

    if "antenv.axon_hooks" in sys.modules:
        return
    try:
        import antenv

        mod = types.ModuleType("antenv.axon_hooks")
        _hook = [None]
        mod.set_axon_ntff_profile_hook = lambda h: _hook.__setitem__(0, h)
        mod.get_axon_ntff_profile_hook = lambda: _hook[0]
        sys.modules["antenv.axon_hooks"] = mod
        antenv.axon_hooks = mod
        from trn_agent_boot.trn_boot import _ntff_profile_via_ctypes

        mod.set_axon_ntff_profile_hook(
            _ntff_profile_via_ctypes("/opt/axon/libaxon_pjrt.so"))
        import concourse.bass_utils as _bu

        _bu.upload_artifacts = lambda d: d  # no artifact store in this container
    except Exception as e:  # profiling is best-effort
        print(f"profile hook setup failed: {e}", file=sys.stderr)


def kernel(x, conv_w, bn2_g, bn2_b, bn2_m, bn2_v, centers, scales,
           bn1_g, bn1_b, bn1_m, bn1_v, head_w, head_b):
    x = np.ascontiguousarray(np.asarray(x, dtype=np.float32))
    wT, prr, packf, small, ssc, bias2 = _host_params(
        np.asarray(conv_w, np.float32), np.asarray(bn2_g, np.float32),
        np.asarray(bn2_b, np.float32), np.asarray(bn2_m, np.float32),
        np.asarray(bn2_v, np.float32), np.asarray(centers, np.float32),
        np.asarray(scales, np.float32), np.asarray(bn1_g, np.float32),
        np.asarray(bn1_b, np.float32), np.asarray(bn1_m, np.float32),
        np.asarray(bn1_v, np.float32), np.asarray(head_w, np.float32),
        np.asarray(head_b, np.float32))
    has_bias2 = bool(np.abs(bias2).max() > 0)
    nc = _get_program(has_bias2)

    shared = {
        "prw": wT, "prr": prr, "packf": packf, "small": small, "ssc": ssc,
    }
    in_maps = [dict(shared, x=x[b].reshape(D, HW)) for b in range(N_CORES)]

    trace = bool(int(os.environ.get("KERNEL_TRACE", "0")))
    kwargs = {}
    if trace:
        _ensure_profile_hook()
        tdir = os.environ.get("KERNEL_TRACE_DIR")
        if tdir:
            os.makedirs(tdir, exist_ok=True)
            kwargs["tmpdir"] = tdir
    res = run_bass_kernel_spmd(nc, in_maps, list(range(N_CORES)), trace=trace, **kwargs)
    if trace:
        kernel.last_exec_time_ns = res.exec_time_ns
        kernel.last_results = res
    out = np.stack([res.results[b]["out"].reshape(D, H, W) for b in range(N_CORES)])
    return out.astype(np.float32)
